# revision 3
# baseline (speedup 1.0000x reference)
"""Trainium2 Bass kernel for nn_Conv_Embedding_1 (GNN message passing).

Computes, given full inputs:
    h   = embeddings @ W + b
    msg = vals[:,None] * h[cols]
    h   = segment_sum(msg, rows, num_segments=N)
    h   = LayerNorm(relu(h)) * gamma + beta
    out = where(1 <= x <= N, h[x-1], 0)

Distribution strategy (8 cores, zero collectives):
  * Nodes padded to 100352 = 8 * 98 * 128; each core owns a contiguous range
    of 12544 nodes (98 blocks of 128).
  * Edges bucketed by destination row block on the host; core k receives
    exactly the edges whose row lands in its range, so its segment-sum is
    complete locally (no all-reduce needed).
  * Algebraic identity  A@(E@W + 1 b^T) = (A@E)@W + (A@1) b^T : the kernel
    gathers raw embedding rows per edge (not h), aggregates via a one-hot
    matmul into PSUM, and applies W / bias / relu / LayerNorm per 128-node
    block after aggregation. The embedding table is extended with a ones
    column on the host so the same matmul accumulates the weighted degree
    A@1 in PSUM column 128.
  * Batch indices bucketed by owning node range on the host; each core
    gathers output rows from its local LayerNorm shard only.
"""

import sys

for _p in ("/opt/trn_rl_repo", "/root/.axon_site/_ro/trn_rl_repo"):
    if _p not in sys.path:
        sys.path.insert(0, _p)

import numpy as np

import concourse.bacc as bacc
import concourse.bass as bass
import concourse.mybir as mybir
import concourse.tile as tile
from concourse.bass_utils import run_bass_kernel_spmd
from concourse.masks import make_identity

P = 128
D = 128
DE = D + 1  # emb row + ones column
N_NODES = 100000
N_CORES = 8
NBLK = 98  # node blocks per core
NODES_PER_CORE = NBLK * P  # 12544
N_PAD = N_CORES * NODES_PER_CORE  # 100352
LN_EPS = 1e-5

_cache = {}


def _build_program(kchunk: int, ncall: int, apply_gb: bool, repeat: int = 1):
    f32 = mybir.dt.float32
    i32 = mybir.dt.int32

    nc = bacc.Bacc(
        "TRN2", target_bir_lowering=False, debug=False, num_devices=N_CORES
    )
    emb = nc.dram_tensor("emb", [N_NODES, DE], f32, kind="ExternalInput").ap()
    weight = nc.dram_tensor("weight", [D, D], f32, kind="ExternalInput").ap()
    biasg = nc.dram_tensor("biasg", [1, D], f32, kind="ExternalInput").ap()
    cols_i = nc.dram_tensor(
        "cols_i", [NBLK, P, kchunk], i32, kind="ExternalInput"
    ).ap()
    rv = nc.dram_tensor(
        "rv", [NBLK, P, 2, kchunk], f32, kind="ExternalInput"
    ).ap()
    xloc = nc.dram_tensor("xloc", [ncall, P], i32, kind="ExternalInput").ap()
    if apply_gb:
        gb = nc.dram_tensor("gb", [2, P, D], f32, kind="ExternalInput").ap()
    out = nc.dram_tensor("out", [ncall * P, D], f32, kind="ExternalOutput").ap()
    h3d = nc.dram_tensor("h3d", [NODES_PER_CORE, D], f32).ap()

    with tile.TileContext(nc) as tc:
        with (
            tc.tile_pool(name="const", bufs=1) as cpool,
            tc.tile_pool(name="slab", bufs=3) as slab,
            tc.tile_pool(name="gath", bufs=3) as gath,
            tc.tile_pool(name="oh", bufs=6) as ohp,
            tc.tile_pool(name="epi", bufs=3) as epi,
            tc.tile_pool(name="psum", bufs=2, space="PSUM") as psum,
            tc.tile_pool(name="fin", bufs=4) as fin,
        ):
            ident = cpool.tile([P, P], f32)
            make_identity(nc, ident[:])
            iota_i = cpool.tile([P, P], i32)
            nc.gpsimd.iota(
                iota_i[:], pattern=[[1, P]], base=0, channel_multiplier=0
            )
            iota_f = cpool.tile([P, P], f32)
            nc.vector.tensor_copy(iota_f[:], iota_i[:])
            w_sb = cpool.tile([P, D], f32)
            nc.sync.dma_start(out=w_sb[:], in_=weight[:, :])
            bias_sb = cpool.tile([1, D], f32)
            nc.sync.dma_start(out=bias_sb[:], in_=biasg[:, :])
            eps_sb = cpool.tile([P, 1], f32)
            nc.vector.memset(eps_sb[:], LN_EPS)
            if apply_gb:
                gamma_sb = cpool.tile([P, D], f32)
                nc.sync.dma_start(out=gamma_sb[:], in_=gb[0, :, :])
                beta_sb = cpool.tile([P, D], f32)
                nc.sync.dma_start(out=beta_sb[:], in_=gb[1, :, :])

            def body(_iv=None):
                for b in range(NBLK):
                    idx_t = slab.tile([P, kchunk], i32, tag="idx")
                    nc.sync.dma_start(out=idx_t[:], in_=cols_i[b, :, :])
                    rv_t = slab.tile([P, 2, kchunk], f32, tag="rv")
                    nc.sync.dma_start(out=rv_t[:], in_=rv[b, :, :, :])

                    g = gath.tile([P, kchunk, DE], f32, tag="g")
                    for j in range(kchunk):
                        nc.gpsimd.indirect_dma_start(
                            out=g[:, j, :],
                            out_offset=None,
                            in_=emb[:, :],
                            in_offset=bass.IndirectOffsetOnAxis(
                                ap=idx_t[:, j : j + 1], axis=0
                            ),
                        )

                    acc = psum.tile([P, DE], f32, tag="acc")
                    for j in range(kchunk):
                        oh = ohp.tile([P, P], f32, tag="oh")
                        nc.vector.tensor_scalar(
                            out=oh[:],
                            in0=iota_f[:],
                            scalar1=rv_t[:, 0, j : j + 1],
                            scalar2=rv_t[:, 1, j : j + 1],
                            op0=mybir.AluOpType.is_equal,
                            op1=mybir.AluOpType.mult,
                        )
                        nc.tensor.matmul(
                            acc[:],
                            lhsT=oh[:],
                            rhs=g[:, j, :],
                            start=(j == 0),
                            stop=(j == kchunk - 1),
                        )

                    agg_sb = epi.tile([P, DE], f32, tag="agg")
                    nc.any.tensor_copy(agg_sb[:], acc[:])
                    aggT_ps = psum.tile([P, P], f32, tag="aggT")
                    nc.tensor.transpose(aggT_ps[:], agg_sb[:, 0:D], ident[:])
                    aggT_sb = epi.tile([P, P], f32, tag="aggTs")
                    nc.any.tensor_copy(aggT_sb[:], aggT_ps[:])
                    degT_ps = psum.tile([1, P], f32, tag="degT")
                    nc.tensor.transpose(
                        degT_ps[:], agg_sb[:, D : D + 1], ident[:]
                    )
                    degT_sb = epi.tile([1, P], f32, tag="degTs")
                    nc.any.tensor_copy(degT_sb[:], degT_ps[:])

                    h2_ps = psum.tile([P, D], f32, tag="h2")
                    nc.tensor.matmul(
                        h2_ps[:], lhsT=aggT_sb[:], rhs=w_sb[:],
                        start=True, stop=False,
                    )
                    nc.tensor.matmul(
                        h2_ps[:], lhsT=degT_sb[:], rhs=bias_sb[:],
                        start=False, stop=True,
                    )

                    h2s = epi.tile([P, D], f32, tag="h2s")
                    nc.scalar.activation(
                        out=h2s[:],
                        in_=h2_ps[:],
                        func=mybir.ActivationFunctionType.Relu,
                        bias=0.0,
                        scale=1.0,
                    )
                    stats = epi.tile([P, 6], f32, tag="stats")
                    nc.vector.bn_stats(stats[:], h2s[:])
                    mv = epi.tile([P, 2], f32, tag="mv")
                    nc.vector.bn_aggr(mv[:], stats[:])
                    nc.scalar.activation(
                        out=mv[:, 1:2],
                        in_=mv[:, 1:2],
                        func=mybir.ActivationFunctionType.Sqrt,
                        bias=eps_sb[:],
                        scale=1.0,
                        alpha=0.0,
                    )
                    nc.vector.reciprocal(out=mv[:, 1:2], in_=mv[:, 1:2])
                    h3_sb = epi.tile([P, D], f32, tag="h3")
                    nc.vector.tensor_scalar(
                        out=h3_sb[:],
                        in0=h2s[:],
                        scalar1=mv[:, 0:1],
                        scalar2=mv[:, 1:2],
                        op0=mybir.AluOpType.subtract,
                        op1=mybir.AluOpType.mult,
                    )
                    if apply_gb:
                        nc.vector.tensor_tensor(
                            out=h3_sb[:], in0=h3_sb[:], in1=gamma_sb[:],
                            op=mybir.AluOpType.mult,
                        )
                        nc.vector.tensor_tensor(
                            out=h3_sb[:], in0=h3_sb[:], in1=beta_sb[:],
                            op=mybir.AluOpType.add,
                        )
                    nc.sync.dma_start(
                        out=h3d[b * P : (b + 1) * P, :], in_=h3_sb[:]
                    )

                tc.strict_bb_all_engine_barrier()

                for c in range(ncall):
                    xt = fin.tile([P, 1], i32, tag="xt")
                    nc.sync.dma_start(out=xt[:], in_=xloc[c, :, None])
                    g2 = fin.tile([P, D], f32, tag="g2")
                    nc.gpsimd.indirect_dma_start(
                        out=g2[:],
                        out_offset=None,
                        in_=h3d[:, :],
                        in_offset=bass.IndirectOffsetOnAxis(
                            ap=xt[:, 0:1], axis=0
                        ),
                    )
                    nc.sync.dma_start(
                        out=out[c * P : (c + 1) * P, :], in_=g2[:]
                    )

            if repeat == 1:
                body()
            else:
                with tc.For_i(0, repeat, 1) as _i:
                    body(_i)

    nc.compile()
    return nc


def _prep(inputs):
    """Host-side sharding: bucket edges by row block, batch indices by node
    range; pad everything to uniform per-core shapes."""
    x = np.asarray(inputs["x"]).astype(np.int64)
    emb_in = np.asarray(inputs["embeddings"], dtype=np.float32)
    emb = np.ones((N_NODES, DE), np.float32)
    emb[:, 0:D] = emb_in
    weight = np.ascontiguousarray(np.asarray(inputs["weight"], dtype=np.float32))
    bias = np.asarray(inputs["bias"], dtype=np.float32).reshape(1, D)
    vals = np.asarray(inputs["vals"], dtype=np.float32)
    rows = np.asarray(inputs["rows"]).astype(np.int64)
    cols = np.asarray(inputs["cols"]).astype(np.int64)
    gamma = np.asarray(inputs["gamma"], dtype=np.float32)
    beta = np.asarray(inputs["beta"], dtype=np.float32)

    apply_gb = not (np.all(gamma == 1.0) and np.all(beta == 0.0))

    # ---- edges: bucket by global node block ----
    gblk = rows >> 7  # row // 128
    order = np.argsort(gblk, kind="stable")
    gblk_s = gblk[order]
    rows_s = rows[order]
    cols_s = cols[order]
    vals_s = vals[order]
    n_gblk = N_PAD // P  # 784
    counts = np.bincount(gblk_s, minlength=n_gblk)
    starts = np.zeros(n_gblk + 1, np.int64)
    np.cumsum(counts, out=starts[1:])
    kchunk = max(1, int(np.ceil(counts.max() / P)))
    ecap = kchunk * P

    cols_a = np.zeros((N_CORES, NBLK, ecap), np.int32)
    rl_a = np.zeros((N_CORES, NBLK, ecap), np.float32)
    v_a = np.zeros((N_CORES, NBLK, ecap), np.float32)
    for g in range(n_gblk):
        s, e = starts[g], starts[g + 1]
        if s == e:
            continue
        k, b = divmod(g, NBLK)
        n = e - s
        cols_a[k, b, :n] = cols_s[s:e]
        rl_a[k, b, :n] = (rows_s[s:e] - g * P).astype(np.float32)
        v_a[k, b, :n] = vals_s[s:e]
    # edge t of a block -> (lane p, chunk j) with p = t % P, j = t // P
    cols_i = cols_a.reshape(N_CORES, NBLK, kchunk, P).transpose(0, 1, 3, 2)
    rl_i = rl_a.reshape(N_CORES, NBLK, kchunk, P).transpose(0, 1, 3, 2)
    v_i = v_a.reshape(N_CORES, NBLK, kchunk, P).transpose(0, 1, 3, 2)
    rv = np.ascontiguousarray(
        np.stack([rl_i, v_i], axis=3)
    )  # [C, NBLK, P, 2, kchunk]
    cols_i = np.ascontiguousarray(cols_i)

    # ---- batch indices: bucket by owning core ----
    valid = (x >= 1) & (x <= N_NODES)
    node = np.where(valid, x - 1, 0)
    core_of = node // NODES_PER_CORE
    pos_lists = []
    cnts = np.zeros(N_CORES, np.int64)
    for k in range(N_CORES):
        pos = np.nonzero(valid & (core_of == k))[0]
        pos_lists.append(pos)
        cnts[k] = len(pos)
    ncall = max(1, int(np.ceil(cnts.max() / P)))
    pbatch = ncall * P
    xloc = np.zeros((N_CORES, ncall, P), np.int32)
    for k in range(N_CORES):
        loc = (node[pos_lists[k]] - k * NODES_PER_CORE).astype(np.int32)
        buf = np.zeros(pbatch, np.int32)
        buf[: cnts[k]] = loc
        xloc[k] = buf.reshape(ncall, P)

    in_maps = []
    for k in range(N_CORES):
        m = {
            "emb": emb,
            "weight": weight,
            "biasg": bias,
            "cols_i": cols_i[k],
            "rv": rv[k],
            "xloc": np.ascontiguousarray(xloc[k]),
        }
        if apply_gb:
            m["gb"] = np.ascontiguousarray(
                np.stack(
                    [
                        np.broadcast_to(gamma, (P, D)),
                        np.broadcast_to(beta, (P, D)),
                    ]
                )
            )
        in_maps.append(m)

    meta = {
        "kchunk": kchunk,
        "ncall": ncall,
        "apply_gb": apply_gb,
        "pos_lists": pos_lists,
        "cnts": cnts,
        "batch": len(x),
    }
    return in_maps, meta


def _assemble(results, meta):
    final = np.zeros((meta["batch"], D), np.float32)
    for k in range(N_CORES):
        cnt = int(meta["cnts"][k])
        if cnt == 0:
            continue
        shard = results[k]["out"]
        final[meta["pos_lists"][k]] = shard[:cnt]
    return final


def get_program(meta, repeat: int = 1):
    key = (meta["kchunk"], meta["ncall"], meta["apply_gb"], repeat)
    if key not in _cache:
        _cache[key] = _build_program(
            meta["kchunk"], meta["ncall"], meta["apply_gb"], repeat
        )
    return _cache[key]


def kernel(**inputs) -> np.ndarray:
    in_maps, meta = _prep(inputs)
    nc = get_program(meta)
    res = run_bass_kernel_spmd(nc, in_maps, list(range(N_CORES)))
    return _assemble(res.results, meta)
